# revision 26
# baseline (speedup 1.0000x reference)
"""Trainium2 Bass kernel for BitNet-style cross-attention (8 NeuronCores).

Strategy: pure data-parallel token sharding. b=2, n=2048 -> 4096 query-token
rows; each of the 8 cores owns 512 of them (cores 0-3 batch 0, 4-7 batch 1)
and computes its output slice fully independently (k/v for the core's batch
are recomputed per core).

All device tensors are feature-major ([dim, tokens]) so no on-chip transposes
are needed; the host supplies transposed views (pure layout transform).

v2 engine assignment (from HW trace analysis): TensorScalar ops with a
runtime per-partition scalar ("TensorScalarPtr") run ~7ns/col on DVE and
~14.6ns/col on GpSimd -- pathologically slow -- while the Act engine applies
pointer scales at ~0.9ns/col and DVE immediates/tensor_tensor run at
~0.4-1ns/col. So:
  - weight quant: DVE abs-sum reduce (mean), Act mul by 1/mean (ptr scale),
    DVE clip-to-[-1.49,1.49] via immediates with int8 rounding output, DVE
    copy int8->bf16.
  - act quant: per-chunk abs_max accumulation (DVE tensor_tensor), one
    gpsimd partition_all_reduce per sub-block (also provides the replicated
    broadcast), reciprocal_approx_fast for the scale, DVE tensor_tensor
    quant+dequant.
  - scores scale (mWq*mWk/sqrt(D)) folded into the Exp activation's scale
    operand; k/v/q evictions are plain DVE copies.
  - softmax normalization: the ones-column in v is pre-scaled by 1/mWv so
    the same reciprocal handles the v weight scale; reciprocal_approx_fast
    + gpsimd partition_broadcast + DVE multiply.
"""

import numpy as np

import concourse.bass as bass
import concourse.mybir as mybir
import concourse.tile as tile
from concourse import bacc, bass_isa
from concourse.bass_utils import run_bass_kernel_spmd

F32 = mybir.dt.float32
BF16 = mybir.dt.bfloat16
I8 = mybir.dt.int8
AX = mybir.AxisListType
OP = mybir.AluOpType
AF = mybir.ActivationFunctionType

P = 128

CFG_FULL = dict(DIM=1024, INNER=1024, H=16, D=64, NTOK=512, MCTX=2048)
N_CORES = 8
EPS = 1e-5


def build(cfg):
    DIM, INNER, H, D = cfg["DIM"], cfg["INNER"], cfg["H"], cfg["D"]
    NTOK, MCTX = cfg["NTOK"], cfg["MCTX"]
    KC = DIM // P          # input-dim chunks
    IC = INNER // P        # inner-dim chunks
    HPC = P // D           # heads per inner chunk (2)
    NKB = MCTX // P        # key blocks
    NTB = NTOK // P        # query-token 128-blocks
    CTB = MCTX // 512 if MCTX >= 512 else 1   # ctx 512-col blocks for k proj
    CW = min(512, MCTX)    # k-proj moving width
    NH = INNER // 512 if INNER >= 512 else 1  # inner 512-halves
    IW = min(512, INNER)
    SUB = min(256, NTOK)   # act-quant token sub-block
    VW = D + 1             # v columns per head incl ones

    nc = bacc.Bacc("TRN2", target_bir_lowering=False, debug=False,
                   num_devices=N_CORES)

    xT = nc.dram_tensor("xT", [DIM, NTOK], F32, kind="ExternalInput")
    cT = nc.dram_tensor("cT", [DIM, MCTX], F32, kind="ExternalInput")
    wT = {}
    for w in ("wq", "wk", "wv", "wo"):
        wT[w] = nc.dram_tensor(w + "T", [DIM, INNER], F32, kind="ExternalInput")
    y_out = nc.dram_tensor("y", [NTOK, DIM], F32, kind="ExternalOutput")

    from contextlib import ExitStack
    with tile.TileContext(nc) as tc, ExitStack() as ctx:
        pp = ctx.enter_context(tc.tile_pool(name="persist", bufs=1))
        smp = ctx.enter_context(tc.tile_pool(name="small", bufs=2))
        wsp = ctx.enter_context(tc.tile_pool(name="wstage", bufs=2))
        wbp = ctx.enter_context(tc.tile_pool(name="wbpool", bufs=2))
        ps_proj = ctx.enter_context(tc.tile_pool(name="ps_proj", bufs=2,
                                                 space="PSUM"))
        ps_sc = ctx.enter_context(tc.tile_pool(name="ps_sc", bufs=2,
                                               space="PSUM"))
        ps_o = ctx.enter_context(tc.tile_pool(name="ps_o", bufs=2,
                                              space="PSUM"))

        # ---- persistent SBUF tensors (live across phases) ----------------
        qb = pp.tile([P, IC * NTOK], BF16, tag="qb")      # q raw, T-major
        kb = pp.tile([P, IC * MCTX], BF16, tag="kb")      # k int, T-major
        vb = pp.tile([P, NKB * H * VW], BF16, tag="vb")   # v deq + 1/mWv col
        invT = pp.tile([P, NKB], F32, tag="invT")         # ctx tok scale cols
        qkinv = pp.tile([P, NKB], F32, tag="qkinv")       # exp scale cols
        ones128 = pp.tile([P, 1], F32, tag="ones128")
        nc.vector.memset(ones128[:], 1.0 / 128.0)

        # ---- weight quantization -----------------------------------------
        wmean = {}

        def quant_weight(w):
            # mean(|w|) pass: Act computes |w| with a free per-partition
            # abs-sum via accum_out (keeps the hot DVE engine clear)
            wpart = smp.tile([P, KC], F32, tag="wpart")
            for c in range(KC):
                s = wsp.tile([P, INNER], F32, tag="wst")
                nc.sync.dma_start(out=s[:], in_=wT[w].ap()[c * P:(c + 1) * P, :])
                wsc = wsp.tile([P, INNER], F32, tag="wsc")
                nc.scalar.activation(wsc[:], s[:], AF.Abs,
                                     accum_out=wpart[:, c:c + 1])
            wsum = smp.tile([P, 1], F32, tag="wsum")
            nc.vector.tensor_reduce(wsum[:], wpart[:], axis=AX.X, op=OP.add)
            wrep = smp.tile([P, 1], F32, tag="wrep")
            nc.gpsimd.partition_all_reduce(wrep[:], wsum[:], channels=P,
                                           reduce_op=bass_isa.ReduceOp.add)
            mean = smp.tile([P, 1], F32, tag="wmean_" + w, name="mean_" + w)
            nc.vector.tensor_scalar(mean[:], wrep[:], 1.0 / (DIM * INNER),
                                    EPS, OP.mult, OP.max)
            qs = smp.tile([P, 1], F32, tag="wqs_" + w, name="qs_" + w)
            nc.vector.reciprocal(qs[:], mean[:])
            wmean[w] = mean
            # ternary pass: Act applies the runtime scale, DVE clips+rounds
            # via int8 output (immediates only), DVE widens to bf16.
            wbt = wbp.tile([P, KC * INNER], BF16, tag="wb", name="wb_" + w)
            for c in range(KC):
                s = wsp.tile([P, INNER], F32, tag="wst")
                nc.sync.dma_start(out=s[:], in_=wT[w].ap()[c * P:(c + 1) * P, :])
                t = wsp.tile([P, INNER], F32, tag="wt")
                nc.vector.tensor_tensor(t[:], s[:],
                                        qs[:].broadcast_to([P, INNER]),
                                        op=OP.mult)
                t8 = wsp.tile([P, INNER], I8, tag="wt8", bufs=1)
                nc.vector.tensor_scalar(t8[:], t[:], 1.49, -1.49,
                                        OP.min, OP.max)
                nc.vector.tensor_copy(wbt[:, c * INNER:(c + 1) * INNER], t8[:])
            return wbt

        # ---- activation quantization (T-major) ---------------------------
        # deq=True: dstT holds dequantized bf16 (i8*inv) -- used for x, whose
        # per-token scale has nowhere cheaper to go.
        # deq=False: dstT holds the raw int8 values widened to bf16 (exact);
        # the per-token scale column for each 128-token block is extracted
        # into invT via a replicated-row transpose matmul, and applied later
        # per-partition (Exp scale for k, Act eviction scale for v).
        def act_quant(srcT, dstT, ncols, asp, s0, s1, deq, invT=None):
            for sblk in range(s0, s1):
                c0 = sblk * SUB
                stage = asp.tile([P, KC, SUB], F32, tag="astage")
                for c in range(KC):
                    nc.sync.dma_start(
                        out=stage[:, c, :],
                        in_=srcT.ap()[c * P:(c + 1) * P, c0:c0 + SUB])
                pam = asp.tile([P, SUB], F32, tag="apam")
                nc.vector.tensor_reduce(
                    pam[:], stage[:].rearrange("p c s -> p s c"),
                    axis=AX.X, op=OP.max, apply_absolute_value=True)
                arep = asp.tile([P, SUB], F32, tag="arep")
                nc.gpsimd.partition_all_reduce(
                    arep[:], pam[:], channels=P,
                    reduce_op=bass_isa.ReduceOp.absmax)
                inv = asp.tile([P, SUB], F32, tag="ainv")
                nc.vector.tensor_scalar(inv[:], arep[:], EPS, 1.0 / 127.0,
                                        OP.max, OP.mult)
                qsc = asp.tile([P, SUB], F32, tag="aqsc")
                nc.vector.reciprocal_approx_fast(qsc[:], inv[:])
                for c in range(KC):
                    i8 = asp.tile([P, SUB], I8, tag="ai8")
                    nc.vector.tensor_tensor(i8[:], stage[:, c, :], qsc[:],
                                            op=OP.mult)
                    if deq:
                        nc.vector.tensor_tensor(
                            dstT[:, c * ncols + c0:c * ncols + c0 + SUB],
                            i8[:], inv[:], op=OP.mult)
                    else:
                        nc.scalar.copy(
                            dstT[:, c * ncols + c0:c * ncols + c0 + SUB],
                            i8[:])
                if invT is not None:
                    # inv is replicated across partitions; out[t,0] =
                    # sum_p inv[p, t]/128 = inv[t] puts token t's scale on
                    # partition t for each 128-token block
                    for jb in range(SUB // P):
                        kbk = (c0 + jb * P) // P
                        psi = ps_o.tile([P, NTOK], F32, tag="po",
                                        name=f"psi{kbk}")
                        nc.tensor.matmul(psi[:, 0:1],
                                         inv[:, jb * P:(jb + 1) * P],
                                         ones128[:], start=True, stop=True)
                        nc.vector.tensor_copy(invT[:, kbk:kbk + 1],
                                              psi[:, 0:1])

        with ExitStack() as phase12:
            adp = phase12.enter_context(tc.tile_pool(name="adpool", bufs=1))
            asp = phase12.enter_context(tc.tile_pool(name="astage", bufs=2))
            xdT = adp.tile([P, KC * NTOK], BF16, tag="xdT")
            cdT = adp.tile([P, KC * MCTX], BF16, tag="cdT")

            # x quant, wq quant, then q projection starts the PE stream early
            act_quant(xT, xdT, NTOK, asp, 0, NTOK // SUB, deq=True)
            wqb = quant_weight("wq")
            for ic in range(IC):
                ps = ps_proj.tile([P, NTOK], F32, tag="pp", name="psq")
                for c in range(KC):
                    nc.tensor.matmul(
                        ps[:],
                        wqb[:, c * INNER + ic * P: c * INNER + (ic + 1) * P],
                        xdT[:, c * NTOK:(c + 1) * NTOK],
                        start=(c == 0), stop=(c == KC - 1))
                nc.vector.tensor_copy(qb[:, ic * NTOK:(ic + 1) * NTOK], ps[:])

            wkb = quant_weight("wk")
            # scores scale mWq*mWk/sqrt(D): folded into the Exp scale operand
            qkmul = smp.tile([P, 1], F32, tag="qkmul")
            nc.vector.tensor_tensor(qkmul[:], wmean["wq"][:], wmean["wk"][:],
                                    op=OP.mult)
            qksc = smp.tile([P, 1], F32, tag="qksc")
            nc.vector.tensor_scalar(qksc[:], qkmul[:], 1.0 / np.sqrt(D), None,
                                    OP.mult)
            # ctx quant interleaved with k projection per 512-col block
            for tb in range(CTB):
                act_quant(cT, cdT, MCTX, asp,
                          tb * (CW // SUB), (tb + 1) * (CW // SUB),
                          deq=False, invT=invT)
                for ic in range(IC):
                    ps = ps_proj.tile([P, CW], F32, tag="pp", name="psk")
                    for c in range(KC):
                        nc.tensor.matmul(
                            ps[:],
                            wkb[:, c * INNER + ic * P: c * INNER + (ic + 1) * P],
                            cdT[:, c * MCTX + tb * CW: c * MCTX + (tb + 1) * CW],
                            start=(c == 0), stop=(c == KC - 1))
                    nc.vector.tensor_copy(
                        kb[:, ic * MCTX + tb * CW: ic * MCTX + (tb + 1) * CW],
                        ps[:])
            wvb = quant_weight("wv")
            vb3 = vb[:].rearrange("p (k h w) -> p k h w", h=H, w=VW)
            # v stays integer-valued in vb; its per-ctx-token dequant scale
            # inv_c rides in the Exp bias (exp(s+ln(inv)) = inv*exp(s)), and
            # the denominator column compensates with 1/(inv_c*mWv) so the
            # same reciprocal yields softmax-normalized, mWv-scaled output.
            rmv = smp.tile([P, 1], F32, tag="rmv")
            nc.vector.reciprocal(rmv[:], wmean["wv"][:])
            lninvT = pp.tile([P, NKB], F32, tag="lninvT")
            nc.scalar.activation(lninvT[:], invT[:], AF.Ln)
            rinvT = smp.tile([P, NKB], F32, tag="rinvT", bufs=1)
            nc.vector.reciprocal_approx_fast(rinvT[:], invT[:])
            rmvT = smp.tile([P, NKB], F32, tag="rmvT", bufs=1)
            nc.vector.tensor_tensor(rmvT[:], rinvT[:],
                                    rmv[:].broadcast_to([P, NKB]), op=OP.mult)
            for kbk in range(NKB):
                nc.vector.tensor_copy(
                    vb3[:, kbk, :, D],
                    rmvT[:, kbk:kbk + 1].broadcast_to([P, H]))
            for kbk in range(NKB):
                for ih in range(NH):
                    ps = ps_proj.tile([P, IW], F32, tag="pp", name="psv")
                    for c in range(KC):
                        nc.tensor.matmul(
                            ps[:],
                            cdT[:, c * MCTX + kbk * P: c * MCTX + (kbk + 1) * P],
                            wvb[:, c * INNER + ih * IW: c * INNER + (ih + 1) * IW],
                            start=(c == 0), stop=(c == KC - 1))
                    hph = IW // D
                    nc.vector.tensor_copy(
                        vb3[:, kbk, ih * hph:(ih + 1) * hph, 0:D],
                        ps[:].rearrange("p (h d) -> p h d", d=D))

        # per-kblock Exp scale: mWq*mWk/sqrt(D) * ctx token scale column
        nc.scalar.mul(qkinv[:], invT[:], qksc[:])

        # wo quant emitted before attention so its DMA/Act/DVE work overlaps
        # the PE-heavy attention phase
        wob = quant_weight("wo")

        # ---- attention ---------------------------------------------------
        op_pool = ctx.enter_context(tc.tile_pool(name="opool", bufs=1))
        otT = op_pool.tile([P, IC * NTOK], F32, tag="otT")
        oqdT = op_pool.tile([P, IC * NTOK], BF16, tag="oqdT")
        ot3 = otT[:].rearrange("p (c t) -> p c t", c=IC)
        omx = op_pool.tile([P, NTOK], F32, tag="omx", bufs=1)
        omn = op_pool.tile([P, NTOK], F32, tag="omn", bufs=1)
        with tc.tile_pool(name="etile", bufs=4) as ep:
            for hp in range(H // 2):
                hA, hB = 2 * hp, 2 * hp + 1
                icA, pA = (hA * D) // P, (hA * D) % P
                icB, pB = (hB * D) // P, (hB * D) % P
                popool, potag = (ps_o, "po") if hp % 2 == 0 else (ps_proj, "pp")
                po = [popool.tile([P, NTOK], F32, tag=potag, name=f"po{hp}_{j}")
                      for j in range(2)]
                for kbk in range(NKB):
                    ss = ps_sc.tile([P, 2, NTOK], F32, tag="ss", name="ss")
                    for j, (h, ich, ph) in enumerate(
                            [(hA, icA, pA), (hB, icB, pB)]):
                        nc.tensor.matmul(
                            ss[:, j, :],
                            kb[ph:ph + D,
                               ich * MCTX + kbk * P: ich * MCTX + (kbk + 1) * P],
                            qb[ph:ph + D, ich * NTOK:(ich + 1) * NTOK],
                            start=True, stop=True)
                    et = ep.tile([P, 2, NTOK], BF16, tag="et")
                    nc.scalar.activation(et[:], ss[:], AF.Exp,
                                         scale=qkinv[:, kbk:kbk + 1],
                                         bias=lninvT[:, kbk:kbk + 1])
                    for j, h in enumerate((hA, hB)):
                        nc.tensor.matmul(
                            po[j][0:VW, :],
                            vb3[:, kbk, h, :],
                            et[:, j, :],
                            start=(kbk == 0), stop=(kbk == NKB - 1))
                for j, (h, ich, ph) in enumerate([(hA, icA, pA), (hB, icB, pB)]):
                    # reciprocal_approx_fast mis-handles inputs at a nonzero
                    # partition offset: stage the denominator row (psum row D)
                    # to a partition-0 tile first
                    dn = op_pool.tile([1, NTOK], F32, tag="dn", bufs=1)
                    nc.vector.tensor_copy(dn[:], po[j][D:D + 1, :])
                    rd = op_pool.tile([1, NTOK], F32, tag="rd", bufs=2)
                    nc.vector.reciprocal_approx_fast(rd[:], dn[:])
                    rb = op_pool.tile([D, NTOK], F32, tag="rb", bufs=2)
                    nc.gpsimd.partition_broadcast(rb[:], rd[:])
                    nc.vector.tensor_tensor(
                        otT[ph:ph + D, ich * NTOK:(ich + 1) * NTOK],
                        po[j][0:D, :], rb[:], op=OP.mult)
                # head-pair hp fills otT chunk hp: fold it into the running
                # per-token max/min while later heads are still computing
                if hp == 0:
                    nc.vector.tensor_copy(omx[:], ot3[:, 0, :])
                    nc.vector.tensor_copy(omn[:], ot3[:, 0, :])
                else:
                    nc.vector.tensor_tensor(omx[:], omx[:],
                                            ot3[:, hp, :], op=OP.max)
                    nc.vector.tensor_tensor(omn[:], omn[:],
                                            ot3[:, hp, :], op=OP.min)

        # ---- attn-out quantization + output projection -------------------
        with tc.tile_pool(name="oq", bufs=2) as oqp, \
                tc.tile_pool(name="ysb", bufs=2) as yp:
            oamax = op_pool.tile([P, NTOK], F32, tag="oamax", bufs=1)
            nc.vector.tensor_scalar(oamax[:], omn[:], -1.0, None, OP.mult)
            nc.vector.tensor_tensor(oamax[:], oamax[:], omx[:], op=OP.max)
            oarep = oqp.tile([P, NTOK], F32, tag="oarep")
            nc.gpsimd.partition_all_reduce(
                oarep[:], oamax[:], channels=P,
                reduce_op=bass_isa.ReduceOp.absmax)
            oinv = op_pool.tile([P, NTOK], F32, tag="oinv", bufs=1)
            nc.vector.tensor_scalar(oinv[:], oarep[:], EPS, 1.0 / 127.0,
                                    OP.max, OP.mult)
            oqsc = op_pool.tile([P, NTOK], F32, tag="oqsc", bufs=1)
            nc.vector.reciprocal_approx_fast(oqsc[:], oinv[:])
            for c in range(IC):
                i8 = oqp.tile([P, NTOK], I8, tag="oi8")
                nc.vector.tensor_tensor(i8[:], ot3[:, c, :], oqsc[:], op=OP.mult)
                nc.vector.tensor_tensor(oqdT[:, c * NTOK:(c + 1) * NTOK],
                                        i8[:], oinv[:], op=OP.mult)

            for tb in range(NTB):
                for oh in range(DIM // IW):
                    ps = ps_proj.tile([P, IW], F32, tag="pp", name="psy")
                    for c in range(IC):
                        nc.tensor.matmul(
                            ps[:],
                            oqdT[:, c * NTOK + tb * P: c * NTOK + (tb + 1) * P],
                            wob[:, c * INNER + oh * IW: c * INNER + (oh + 1) * IW],
                            start=(c == 0), stop=(c == IC - 1))
                    ysb = yp.tile([P, IW], F32, tag="ysb")
                    nc.scalar.mul(ysb[:], ps[:], wmean["wo"][:])
                    nc.sync.dma_start(
                        out=y_out.ap()[tb * P:(tb + 1) * P,
                                       oh * IW:(oh + 1) * IW],
                        in_=ysb[:])
    nc.compile()
    return nc


_CACHE = {}


def _get_nc(key, cfg):
    if key not in _CACHE:
        _CACHE[key] = build(cfg)
    return _CACHE[key]


def _shard(x, context, wq, wk, wv, wo, NTOK):
    b = x.shape[0]
    wmaps = {w + "T": np.ascontiguousarray(a.T)
             for w, a in (("wq", wq), ("wk", wk), ("wv", wv), ("wo", wo))}
    cores_per_b = N_CORES // b
    in_maps = []
    for core in range(N_CORES):
        bi = core // cores_per_b
        t0 = (core % cores_per_b) * NTOK
        in_maps.append(dict(
            xT=np.ascontiguousarray(x[bi, t0:t0 + NTOK, :].T),
            cT=np.ascontiguousarray(context[bi].T),
            **wmaps))
    return in_maps


def _assemble(results, b, n, dim, NTOK):
    out = np.empty((b, n, dim), dtype=np.float32)
    cores_per_b = N_CORES // b
    for core in range(N_CORES):
        bi = core // cores_per_b
        t0 = (core % cores_per_b) * NTOK
        out[bi, t0:t0 + NTOK, :] = results[core]["y"]
    return out


def run(x, context, wq, wk, wv, wo, trace=False):
    cfg = CFG_FULL
    b, n, dim = x.shape
    NTOK = cfg["NTOK"]
    nc = _get_nc("full", cfg)
    in_maps = _shard(x, context, wq, wk, wv, wo, NTOK)
    res = run_bass_kernel_spmd(nc, in_maps, list(range(N_CORES)), trace=trace)
    return _assemble(res.results, b, n, dim, NTOK), res


def kernel(x, context, wq, wk, wv, wo):
    return run(x, context, wq, wk, wv, wo, trace=False)[0]


if __name__ == "__main__":
    ins = {k: np.random.randn(*s).astype(np.float32) * (0.02 if k[0] == 'w' else 1.0)
           for k, s in [("x", (2, 2048, 1024)), ("context", (2, 2048, 1024)),
                        ("wq", (1024, 1024)), ("wk", (1024, 1024)),
                        ("wv", (1024, 1024)), ("wo", (1024, 1024))]}
    y = kernel(**ins)
    print("kernel output", y.shape, y.dtype, np.abs(y).max())


# revision 27
# speedup vs baseline: 1.0178x; 1.0178x over previous
"""Trainium2 Bass kernel for BitNet-style cross-attention (8 NeuronCores).

Strategy: pure data-parallel token sharding. b=2, n=2048 -> 4096 query-token
rows; each of the 8 cores owns 512 of them (cores 0-3 batch 0, 4-7 batch 1)
and computes its output slice fully independently (k/v for the core's batch
are recomputed per core).

All device tensors are feature-major ([dim, tokens]) so no on-chip transposes
are needed; the host supplies transposed views (pure layout transform).

v2 engine assignment (from HW trace analysis): TensorScalar ops with a
runtime per-partition scalar ("TensorScalarPtr") run ~7ns/col on DVE and
~14.6ns/col on GpSimd -- pathologically slow -- while the Act engine applies
pointer scales at ~0.9ns/col and DVE immediates/tensor_tensor run at
~0.4-1ns/col. So:
  - weight quant: DVE abs-sum reduce (mean), Act mul by 1/mean (ptr scale),
    DVE clip-to-[-1.49,1.49] via immediates with int8 rounding output, DVE
    copy int8->bf16.
  - act quant: per-chunk abs_max accumulation (DVE tensor_tensor), one
    gpsimd partition_all_reduce per sub-block (also provides the replicated
    broadcast), reciprocal_approx_fast for the scale, DVE tensor_tensor
    quant+dequant.
  - scores scale (mWq*mWk/sqrt(D)) folded into the Exp activation's scale
    operand; k/v/q evictions are plain DVE copies.
  - softmax normalization: the ones-column in v is pre-scaled by 1/mWv so
    the same reciprocal handles the v weight scale; reciprocal_approx_fast
    + gpsimd partition_broadcast + DVE multiply.
"""

import numpy as np

import concourse.bass as bass
import concourse.mybir as mybir
import concourse.tile as tile
from concourse import bacc, bass_isa
from concourse.bass_utils import run_bass_kernel_spmd

F32 = mybir.dt.float32
BF16 = mybir.dt.bfloat16
I8 = mybir.dt.int8
AX = mybir.AxisListType
OP = mybir.AluOpType
AF = mybir.ActivationFunctionType

P = 128

CFG_FULL = dict(DIM=1024, INNER=1024, H=16, D=64, NTOK=512, MCTX=2048)
N_CORES = 8
EPS = 1e-5


def build(cfg):
    DIM, INNER, H, D = cfg["DIM"], cfg["INNER"], cfg["H"], cfg["D"]
    NTOK, MCTX = cfg["NTOK"], cfg["MCTX"]
    KC = DIM // P          # input-dim chunks
    IC = INNER // P        # inner-dim chunks
    HPC = P // D           # heads per inner chunk (2)
    NKB = MCTX // P        # key blocks
    NTB = NTOK // P        # query-token 128-blocks
    CTB = MCTX // 512 if MCTX >= 512 else 1   # ctx 512-col blocks for k proj
    CW = min(512, MCTX)    # k-proj moving width
    NH = INNER // 512 if INNER >= 512 else 1  # inner 512-halves
    IW = min(512, INNER)
    SUB = min(256, NTOK)   # act-quant token sub-block
    VW = D + 1             # v columns per head incl ones

    nc = bacc.Bacc("TRN2", target_bir_lowering=False, debug=False,
                   num_devices=N_CORES)

    xT = nc.dram_tensor("xT", [DIM, NTOK], F32, kind="ExternalInput")
    cT = nc.dram_tensor("cT", [DIM, MCTX], F32, kind="ExternalInput")
    wT = {}
    for w in ("wq", "wk", "wv", "wo"):
        wT[w] = nc.dram_tensor(w + "T", [DIM, INNER], F32, kind="ExternalInput")
    y_out = nc.dram_tensor("y", [NTOK, DIM], F32, kind="ExternalOutput")

    from contextlib import ExitStack
    with tile.TileContext(nc) as tc, ExitStack() as ctx:
        pp = ctx.enter_context(tc.tile_pool(name="persist", bufs=1))
        smp = ctx.enter_context(tc.tile_pool(name="small", bufs=2))
        wsp = ctx.enter_context(tc.tile_pool(name="wstage", bufs=2))
        wbp = ctx.enter_context(tc.tile_pool(name="wbpool", bufs=2))
        ps_proj = ctx.enter_context(tc.tile_pool(name="ps_proj", bufs=2,
                                                 space="PSUM"))
        ps_sc = ctx.enter_context(tc.tile_pool(name="ps_sc", bufs=2,
                                               space="PSUM"))
        ps_o = ctx.enter_context(tc.tile_pool(name="ps_o", bufs=2,
                                              space="PSUM"))

        # ---- persistent SBUF tensors (live across phases) ----------------
        qb = pp.tile([P, IC * NTOK], BF16, tag="qb")      # q raw, T-major
        kb = pp.tile([P, IC * MCTX], BF16, tag="kb")      # k int, T-major
        vb = pp.tile([P, NKB * H * VW], BF16, tag="vb")   # v deq + 1/mWv col
        invT = pp.tile([P, NKB], F32, tag="invT")         # ctx tok scale cols
        qkinv = pp.tile([P, NKB], F32, tag="qkinv")       # exp scale cols
        ones128 = pp.tile([P, 1], F32, tag="ones128")
        nc.vector.memset(ones128[:], 1.0 / 128.0)

        # ---- weight quantization -----------------------------------------
        wmean = {}

        def quant_weight(w):
            # mean(|w|) pass: Act computes |w| with a free per-partition
            # abs-sum via accum_out (keeps the hot DVE engine clear)
            wpart = smp.tile([P, KC], F32, tag="wpart")
            for c in range(KC):
                s = wsp.tile([P, INNER], F32, tag="wst")
                nc.sync.dma_start(out=s[:], in_=wT[w].ap()[c * P:(c + 1) * P, :])
                wsc = wsp.tile([P, INNER], F32, tag="wsc")
                nc.scalar.activation(wsc[:], s[:], AF.Abs,
                                     accum_out=wpart[:, c:c + 1])
            wsum = smp.tile([P, 1], F32, tag="wsum")
            nc.vector.tensor_reduce(wsum[:], wpart[:], axis=AX.X, op=OP.add)
            wrep = smp.tile([P, 1], F32, tag="wrep")
            nc.gpsimd.partition_all_reduce(wrep[:], wsum[:], channels=P,
                                           reduce_op=bass_isa.ReduceOp.add)
            mean = smp.tile([P, 1], F32, tag="wmean_" + w, name="mean_" + w)
            nc.vector.tensor_scalar(mean[:], wrep[:], 1.0 / (DIM * INNER),
                                    EPS, OP.mult, OP.max)
            qs = smp.tile([P, 1], F32, tag="wqs_" + w, name="qs_" + w)
            nc.vector.reciprocal(qs[:], mean[:])
            wmean[w] = mean
            # ternary pass: Act applies the runtime scale, DVE clips+rounds
            # via int8 output (immediates only), DVE widens to bf16.
            wbt = wbp.tile([P, KC * INNER], BF16, tag="wb", name="wb_" + w)
            for c in range(KC):
                s = wsp.tile([P, INNER], F32, tag="wst")
                nc.sync.dma_start(out=s[:], in_=wT[w].ap()[c * P:(c + 1) * P, :])
                # Act rounds on int8 output (round-half-even, matches
                # jnp.round); |w*qs| < 127 so no saturation happens here and
                # the ternary clip to [-1,1] rides the DVE widening cast
                t8 = wsp.tile([P, INNER], I8, tag="wt8")
                nc.scalar.mul(t8[:], s[:], qs[:])
                nc.vector.tensor_scalar(wbt[:, c * INNER:(c + 1) * INNER],
                                        t8[:], 1.0, -1.0, OP.min, OP.max)
            return wbt

        # ---- activation quantization (T-major) ---------------------------
        # deq=True: dstT holds dequantized bf16 (i8*inv) -- used for x, whose
        # per-token scale has nowhere cheaper to go.
        # deq=False: dstT holds the raw int8 values widened to bf16 (exact);
        # the per-token scale column for each 128-token block is extracted
        # into invT via a replicated-row transpose matmul, and applied later
        # per-partition (Exp scale for k, Act eviction scale for v).
        def act_quant(srcT, dstT, ncols, asp, s0, s1, deq, invT=None):
            for sblk in range(s0, s1):
                c0 = sblk * SUB
                stage = asp.tile([P, KC, SUB], F32, tag="astage")
                for c in range(KC):
                    nc.sync.dma_start(
                        out=stage[:, c, :],
                        in_=srcT.ap()[c * P:(c + 1) * P, c0:c0 + SUB])
                pam = asp.tile([P, SUB], F32, tag="apam")
                nc.vector.tensor_reduce(
                    pam[:], stage[:].rearrange("p c s -> p s c"),
                    axis=AX.X, op=OP.max, apply_absolute_value=True)
                arep = asp.tile([P, SUB], F32, tag="arep")
                nc.gpsimd.partition_all_reduce(
                    arep[:], pam[:], channels=P,
                    reduce_op=bass_isa.ReduceOp.absmax)
                inv = asp.tile([P, SUB], F32, tag="ainv")
                nc.vector.tensor_scalar(inv[:], arep[:], EPS, 1.0 / 127.0,
                                        OP.max, OP.mult)
                qsc = asp.tile([P, SUB], F32, tag="aqsc")
                nc.vector.reciprocal_approx_fast(qsc[:], inv[:])
                for c in range(KC):
                    i8 = asp.tile([P, SUB], I8, tag="ai8")
                    nc.vector.tensor_tensor(i8[:], stage[:, c, :], qsc[:],
                                            op=OP.mult)
                    if deq:
                        nc.vector.tensor_tensor(
                            dstT[:, c * ncols + c0:c * ncols + c0 + SUB],
                            i8[:], inv[:], op=OP.mult)
                    else:
                        nc.scalar.copy(
                            dstT[:, c * ncols + c0:c * ncols + c0 + SUB],
                            i8[:])
                if invT is not None:
                    # inv is replicated across partitions; out[t,0] =
                    # sum_p inv[p, t]/128 = inv[t] puts token t's scale on
                    # partition t for each 128-token block
                    for jb in range(SUB // P):
                        kbk = (c0 + jb * P) // P
                        psi = ps_o.tile([P, NTOK], F32, tag="po",
                                        name=f"psi{kbk}")
                        nc.tensor.matmul(psi[:, 0:1],
                                         inv[:, jb * P:(jb + 1) * P],
                                         ones128[:], start=True, stop=True)
                        nc.vector.tensor_copy(invT[:, kbk:kbk + 1],
                                              psi[:, 0:1])

        with ExitStack() as phase12:
            adp = phase12.enter_context(tc.tile_pool(name="adpool", bufs=1))
            asp = phase12.enter_context(tc.tile_pool(name="astage", bufs=2))
            xdT = adp.tile([P, KC * NTOK], BF16, tag="xdT")
            cdT = adp.tile([P, KC * MCTX], BF16, tag="cdT")

            # x quant, wq quant, then q projection starts the PE stream early
            act_quant(xT, xdT, NTOK, asp, 0, NTOK // SUB, deq=True)
            wqb = quant_weight("wq")
            for ic in range(IC):
                ps = ps_proj.tile([P, NTOK], F32, tag="pp", name="psq")
                for c in range(KC):
                    nc.tensor.matmul(
                        ps[:],
                        wqb[:, c * INNER + ic * P: c * INNER + (ic + 1) * P],
                        xdT[:, c * NTOK:(c + 1) * NTOK],
                        start=(c == 0), stop=(c == KC - 1))
                nc.vector.tensor_copy(qb[:, ic * NTOK:(ic + 1) * NTOK], ps[:])

            wkb = quant_weight("wk")
            # scores scale mWq*mWk/sqrt(D): folded into the Exp scale operand
            qkmul = smp.tile([P, 1], F32, tag="qkmul")
            nc.vector.tensor_tensor(qkmul[:], wmean["wq"][:], wmean["wk"][:],
                                    op=OP.mult)
            qksc = smp.tile([P, 1], F32, tag="qksc")
            nc.vector.tensor_scalar(qksc[:], qkmul[:], 1.0 / np.sqrt(D), None,
                                    OP.mult)
            # ctx quant interleaved with k projection per 512-col block
            for tb in range(CTB):
                act_quant(cT, cdT, MCTX, asp,
                          tb * (CW // SUB), (tb + 1) * (CW // SUB),
                          deq=False, invT=invT)
                for ic in range(IC):
                    ps = ps_proj.tile([P, CW], F32, tag="pp", name="psk")
                    for c in range(KC):
                        nc.tensor.matmul(
                            ps[:],
                            wkb[:, c * INNER + ic * P: c * INNER + (ic + 1) * P],
                            cdT[:, c * MCTX + tb * CW: c * MCTX + (tb + 1) * CW],
                            start=(c == 0), stop=(c == KC - 1))
                    nc.vector.tensor_copy(
                        kb[:, ic * MCTX + tb * CW: ic * MCTX + (tb + 1) * CW],
                        ps[:])
            wvb = quant_weight("wv")
            vb3 = vb[:].rearrange("p (k h w) -> p k h w", h=H, w=VW)
            # v stays integer-valued in vb; its per-ctx-token dequant scale
            # inv_c rides in the Exp bias (exp(s+ln(inv)) = inv*exp(s)), and
            # the denominator column compensates with 1/(inv_c*mWv) so the
            # same reciprocal yields softmax-normalized, mWv-scaled output.
            rmv = smp.tile([P, 1], F32, tag="rmv")
            nc.vector.reciprocal(rmv[:], wmean["wv"][:])
            lninvT = pp.tile([P, NKB], F32, tag="lninvT")
            nc.scalar.activation(lninvT[:], invT[:], AF.Ln)
            rinvT = smp.tile([P, NKB], F32, tag="rinvT", bufs=1)
            nc.vector.reciprocal_approx_fast(rinvT[:], invT[:])
            rmvT = smp.tile([P, NKB], F32, tag="rmvT", bufs=1)
            nc.vector.tensor_tensor(rmvT[:], rinvT[:],
                                    rmv[:].broadcast_to([P, NKB]), op=OP.mult)
            for kbk in range(NKB):
                nc.vector.tensor_copy(
                    vb3[:, kbk, :, D],
                    rmvT[:, kbk:kbk + 1].broadcast_to([P, H]))
            for kbk in range(NKB):
                for ih in range(NH):
                    ps = ps_proj.tile([P, IW], F32, tag="pp", name="psv")
                    for c in range(KC):
                        nc.tensor.matmul(
                            ps[:],
                            cdT[:, c * MCTX + kbk * P: c * MCTX + (kbk + 1) * P],
                            wvb[:, c * INNER + ih * IW: c * INNER + (ih + 1) * IW],
                            start=(c == 0), stop=(c == KC - 1))
                    hph = IW // D
                    nc.vector.tensor_copy(
                        vb3[:, kbk, ih * hph:(ih + 1) * hph, 0:D],
                        ps[:].rearrange("p (h d) -> p h d", d=D))

        # per-kblock Exp scale: mWq*mWk/sqrt(D) * ctx token scale column
        nc.scalar.mul(qkinv[:], invT[:], qksc[:])

        # wo quant emitted before attention so its DMA/Act/DVE work overlaps
        # the PE-heavy attention phase
        wob = quant_weight("wo")

        # ---- attention ---------------------------------------------------
        op_pool = ctx.enter_context(tc.tile_pool(name="opool", bufs=1))
        otT = op_pool.tile([P, IC * NTOK], F32, tag="otT")
        oqdT = op_pool.tile([P, IC * NTOK], BF16, tag="oqdT")
        ot3 = otT[:].rearrange("p (c t) -> p c t", c=IC)
        omx = op_pool.tile([P, NTOK], F32, tag="omx", bufs=1)
        omn = op_pool.tile([P, NTOK], F32, tag="omn", bufs=1)
        with tc.tile_pool(name="etile", bufs=4) as ep:
            for hp in range(H // 2):
                hA, hB = 2 * hp, 2 * hp + 1
                icA, pA = (hA * D) // P, (hA * D) % P
                icB, pB = (hB * D) // P, (hB * D) % P
                popool, potag = (ps_o, "po") if hp % 2 == 0 else (ps_proj, "pp")
                po = [popool.tile([P, NTOK], F32, tag=potag, name=f"po{hp}_{j}")
                      for j in range(2)]
                for kbk in range(NKB):
                    ss = ps_sc.tile([P, 2, NTOK], F32, tag="ss", name="ss")
                    for j, (h, ich, ph) in enumerate(
                            [(hA, icA, pA), (hB, icB, pB)]):
                        nc.tensor.matmul(
                            ss[:, j, :],
                            kb[ph:ph + D,
                               ich * MCTX + kbk * P: ich * MCTX + (kbk + 1) * P],
                            qb[ph:ph + D, ich * NTOK:(ich + 1) * NTOK],
                            start=True, stop=True)
                    et = ep.tile([P, 2, NTOK], BF16, tag="et")
                    nc.scalar.activation(et[:], ss[:], AF.Exp,
                                         scale=qkinv[:, kbk:kbk + 1],
                                         bias=lninvT[:, kbk:kbk + 1])
                    for j, h in enumerate((hA, hB)):
                        nc.tensor.matmul(
                            po[j][0:VW, :],
                            vb3[:, kbk, h, :],
                            et[:, j, :],
                            start=(kbk == 0), stop=(kbk == NKB - 1))
                for j, (h, ich, ph) in enumerate([(hA, icA, pA), (hB, icB, pB)]):
                    # reciprocal_approx_fast mis-handles inputs at a nonzero
                    # partition offset: stage the denominator row (psum row D)
                    # to a partition-0 tile first
                    dn = op_pool.tile([1, NTOK], F32, tag="dn", bufs=1)
                    nc.vector.tensor_copy(dn[:], po[j][D:D + 1, :])
                    rd = op_pool.tile([1, NTOK], F32, tag="rd", bufs=2)
                    nc.vector.reciprocal_approx_fast(rd[:], dn[:])
                    rb = op_pool.tile([D, NTOK], F32, tag="rb", bufs=2)
                    nc.gpsimd.partition_broadcast(rb[:], rd[:])
                    nc.vector.tensor_tensor(
                        otT[ph:ph + D, ich * NTOK:(ich + 1) * NTOK],
                        po[j][0:D, :], rb[:], op=OP.mult)
                # head-pair hp fills otT chunk hp: fold it into the running
                # per-token max/min while later heads are still computing
                if hp == 0:
                    nc.vector.tensor_copy(omx[:], ot3[:, 0, :])
                    nc.vector.tensor_copy(omn[:], ot3[:, 0, :])
                else:
                    nc.vector.tensor_tensor(omx[:], omx[:],
                                            ot3[:, hp, :], op=OP.max)
                    nc.vector.tensor_tensor(omn[:], omn[:],
                                            ot3[:, hp, :], op=OP.min)

        # ---- attn-out quantization + output projection -------------------
        with tc.tile_pool(name="oq", bufs=2) as oqp, \
                tc.tile_pool(name="ysb", bufs=2) as yp:
            oamax = op_pool.tile([P, NTOK], F32, tag="oamax", bufs=1)
            nc.vector.tensor_scalar(oamax[:], omn[:], -1.0, None, OP.mult)
            nc.vector.tensor_tensor(oamax[:], oamax[:], omx[:], op=OP.max)
            oarep = oqp.tile([P, NTOK], F32, tag="oarep")
            nc.gpsimd.partition_all_reduce(
                oarep[:], oamax[:], channels=P,
                reduce_op=bass_isa.ReduceOp.absmax)
            oinv = op_pool.tile([P, NTOK], F32, tag="oinv", bufs=1)
            nc.vector.tensor_scalar(oinv[:], oarep[:], EPS, 1.0 / 127.0,
                                    OP.max, OP.mult)
            oqsc = op_pool.tile([P, NTOK], F32, tag="oqsc", bufs=1)
            nc.vector.reciprocal_approx_fast(oqsc[:], oinv[:])
            for c in range(IC):
                i8 = oqp.tile([P, NTOK], I8, tag="oi8")
                nc.vector.tensor_tensor(i8[:], ot3[:, c, :], oqsc[:], op=OP.mult)
                nc.vector.tensor_tensor(oqdT[:, c * NTOK:(c + 1) * NTOK],
                                        i8[:], oinv[:], op=OP.mult)

            for tb in range(NTB):
                for oh in range(DIM // IW):
                    ps = ps_proj.tile([P, IW], F32, tag="pp", name="psy")
                    for c in range(IC):
                        nc.tensor.matmul(
                            ps[:],
                            oqdT[:, c * NTOK + tb * P: c * NTOK + (tb + 1) * P],
                            wob[:, c * INNER + oh * IW: c * INNER + (oh + 1) * IW],
                            start=(c == 0), stop=(c == IC - 1))
                    ysb = yp.tile([P, IW], F32, tag="ysb")
                    nc.scalar.mul(ysb[:], ps[:], wmean["wo"][:])
                    nc.sync.dma_start(
                        out=y_out.ap()[tb * P:(tb + 1) * P,
                                       oh * IW:(oh + 1) * IW],
                        in_=ysb[:])
    nc.compile()
    return nc


_CACHE = {}


def _get_nc(key, cfg):
    if key not in _CACHE:
        _CACHE[key] = build(cfg)
    return _CACHE[key]


def _shard(x, context, wq, wk, wv, wo, NTOK):
    b = x.shape[0]
    wmaps = {w + "T": np.ascontiguousarray(a.T)
             for w, a in (("wq", wq), ("wk", wk), ("wv", wv), ("wo", wo))}
    cores_per_b = N_CORES // b
    in_maps = []
    for core in range(N_CORES):
        bi = core // cores_per_b
        t0 = (core % cores_per_b) * NTOK
        in_maps.append(dict(
            xT=np.ascontiguousarray(x[bi, t0:t0 + NTOK, :].T),
            cT=np.ascontiguousarray(context[bi].T),
            **wmaps))
    return in_maps


def _assemble(results, b, n, dim, NTOK):
    out = np.empty((b, n, dim), dtype=np.float32)
    cores_per_b = N_CORES // b
    for core in range(N_CORES):
        bi = core // cores_per_b
        t0 = (core % cores_per_b) * NTOK
        out[bi, t0:t0 + NTOK, :] = results[core]["y"]
    return out


def run(x, context, wq, wk, wv, wo, trace=False):
    cfg = CFG_FULL
    b, n, dim = x.shape
    NTOK = cfg["NTOK"]
    nc = _get_nc("full", cfg)
    in_maps = _shard(x, context, wq, wk, wv, wo, NTOK)
    res = run_bass_kernel_spmd(nc, in_maps, list(range(N_CORES)), trace=trace)
    return _assemble(res.results, b, n, dim, NTOK), res


def kernel(x, context, wq, wk, wv, wo):
    return run(x, context, wq, wk, wv, wo, trace=False)[0]


if __name__ == "__main__":
    ins = {k: np.random.randn(*s).astype(np.float32) * (0.02 if k[0] == 'w' else 1.0)
           for k, s in [("x", (2, 2048, 1024)), ("context", (2, 2048, 1024)),
                        ("wq", (1024, 1024)), ("wk", (1024, 1024)),
                        ("wv", (1024, 1024)), ("wo", (1024, 1024))]}
    y = kernel(**ins)
    print("kernel output", y.shape, y.dtype, np.abs(y).max())
